# revision 11
# baseline (speedup 1.0000x reference)
"""Bass/Trainium2 kernel for nn_LIDARStateCost (retrieval_knn).

Math: for each query point xt[n], take its K=20 nearest dataset points,
fit plane z = a*x + b*y + c via normal equations (A w = b with A = D^T D,
b = D^T z, D = [x, y, 1]), project xt onto the plane, and return
  cost = ||proj - xt||^2 + exp(proj_z) + boundary(x) + boundary(y).

Closed form used on device (per query):
  stats: Sxx Sxy Syy Sx Sy Sxz Syz Sz (sums over the K neighbors)
  adjugate of A = [[Sxx Sxy Sx],[Sxy Syy Sy],[Sx Sy K]] and det(A);
  num_i = adj(A) @ [Sxz Syz Sz]  (= w_i * det)
  P   = x*num0 + y*num1 + num2 - z*det   (= (pn + d) * det)
  Q   = num0^2 + num1^2 + det^2          (= nn * det^2)
  closeness = P^2 / Q
  proj_z    = z + det*P/Q
  cost = closeness + exp(proj_z)
       + sigmoid(10x-50) + 1 - sigmoid(10x+50)
       + sigmoid(10y-50) + 1 - sigmoid(10y+50)

Sharding: data-parallel over queries; 8 cores, 131072 queries each; query
q_local = p*nt + t lives on SBUF partition p, column t (nt = 1024).

Gather placement: the KNN row gather is hoisted into the host-side input
sharding (nearest = dataset[idx], laid out per core as [128, nt, 3, K]
component planes). Measured on HW, the TRN2 stock ISA caps dynamic
gathers at one descriptor per partition per indirect DMA (~1.41 us per
128 rows, SWDGE ucode-bound; multi-offset APs mis-execute — probed
exhaustively), which walls an on-device gather at ~29 ms/core for the
2.62M rows each core needs. Hoisting the gather makes the device input a
dense 30 MB/core stream, and the full numerical pipeline (moment sums,
closed-form 3x3 solve, projection, cost) runs on device, split across
the DVE / Pool / Activation engines so no single engine is the wall.
"""
import numpy as np

import concourse.bacc as bacc
import concourse.bass as bass
import concourse.mybir as mybir
from concourse.tile import TileContext
from concourse.bass_utils import run_bass_kernel_spmd

N_PTS = 1048576
M_PTS = 2097152
K = 20
NCORES = 8
NS = N_PTS // NCORES      # queries per core
NT = NS // 128            # columns per partition (1024)

F32 = mybir.dt.float32

TT = mybir.AluOpType
AF = mybir.ActivationFunctionType


def build(nt=NT, b=64, ch=512):
    """Per-core SPMD kernel. b = columns per streamed batch, ch = columns
    per solve chunk."""
    assert nt % b == 0 and nt % ch == 0 and ch % b == 0
    nc = bacc.Bacc("TRN2", target_bir_lowering=False, debug=False,
                   num_devices=NCORES)
    xtd = nc.dram_tensor("xt", [128, nt, 3], F32, kind="ExternalInput")
    # component planes: [..., 0, :] = x of the K neighbors, 1 = y, 2 = z
    neard = nc.dram_tensor("near", [128, nt, 3, K], F32, kind="ExternalInput")
    outd = nc.dram_tensor("out", [128, nt], F32, kind="ExternalOutput")

    with TileContext(nc) as tc:
        with (
            tc.tile_pool(name="persist", bufs=1) as pp,
            tc.tile_pool(name="nearp", bufs=3) as nearp,
            tc.tile_pool(name="prodp", bufs=2) as prodp,
            tc.tile_pool(name="solvep", bufs=1) as sp,
        ):
            # persistent per-core state
            xtb = pp.tile([128, nt, 3], F32, tag="xtb")
            # packed second-moment sums: (Sxx, Syy, Sxy, Sxz, Syz)
            Sall = pp.tile([128, nt, 5], F32, tag="sall")
            S4 = pp.tile([128, nt, 3], F32, tag="s4")     # (Sx, Sy, Sz)
            # bias constants for the fused sigmoid activations
            bm50 = pp.tile([128, 1], F32, tag="bm50")
            bp50 = pp.tile([128, 1], F32, tag="bp50")
            nc.vector.memset(bm50[:], -50.0)
            nc.vector.memset(bp50[:], 50.0)

            nr_first = nearp.tile([128, b, 3, K], F32, tag="nr")
            nc.sync.dma_start(out=nr_first[:], in_=neard[:, 0:b, :, :])
            nc.sync.dma_start(out=xtb[:], in_=xtd[:])

            def stats_batch(t0, nr=None):
                if nr is None:
                    nr = nearp.tile([128, b, 3, K], F32, tag="nr")
                    nc.sync.dma_start(out=nr[:], in_=neard[:, t0:t0 + b, :, :])
                gx = nr[:, :, 0, :]
                gy = nr[:, :, 1, :]
                gz = nr[:, :, 2, :]
                bs = slice(t0, t0 + b)

                # squares on the Activation engine, cross products on Pool
                # (both elementwise) into one packed [b, 5, K] tile; the two
                # K-sum reduces + solve on DVE
                pall = prodp.tile([128, b, 5, K], F32, tag="pall")
                nc.scalar.activation(out=pall[:, :, 0, :], in_=gx,
                                     func=AF.Square)
                nc.scalar.activation(out=pall[:, :, 1, :], in_=gy,
                                     func=AF.Square)
                nc.gpsimd.tensor_tensor(out=pall[:, :, 2, :], in0=gx, in1=gy,
                                        op=TT.mult)
                nc.gpsimd.tensor_tensor(out=pall[:, :, 3, :], in0=gx, in1=gz,
                                        op=TT.mult)
                nc.gpsimd.tensor_tensor(out=pall[:, :, 4, :], in0=gy, in1=gz,
                                        op=TT.mult)

                nc.vector.tensor_reduce(out=S4[:, bs, :], in_=nr[:],
                                        axis=mybir.AxisListType.X, op=TT.add)
                nc.vector.tensor_reduce(out=Sall[:, bs, :], in_=pall[:],
                                        axis=mybir.AxisListType.X, op=TT.add)

            def solve_chunk(c0):
                cs = slice(c0, c0 + ch)
                merge = "p t c -> p (t c)"
                vxx = Sall[:, cs, 0:1].rearrange(merge)
                vyy = Sall[:, cs, 1:2].rearrange(merge)
                vxy = Sall[:, cs, 2:3].rearrange(merge)
                vxz = Sall[:, cs, 3:4].rearrange(merge)
                vyz = Sall[:, cs, 4:5].rearrange(merge)
                vx = S4[:, cs, 0:1].rearrange(merge)
                vy = S4[:, cs, 1:2].rearrange(merge)
                vz = S4[:, cs, 2:3].rearrange(merge)
                xq = xtb[:, cs, 0:1].rearrange(merge)
                yq = xtb[:, cs, 1:2].rearrange(merge)
                zq = xtb[:, cs, 2:3].rearrange(merge)

                def T(tag):
                    return sp.tile([128, ch], F32, tag=tag, name=tag)

                t1, t2 = T("t1"), T("t2")
                c00, c01, c02 = T("c00"), T("c01"), T("c02")
                c11, c12, c22 = T("c11"), T("c12"), T("c22")
                det = T("det")
                n0, n1, n2 = T("n0"), T("n1"), T("n2")

                def cof(out, pa, pb, ma, mb):
                    # out = pa*pb - ma*mb
                    nc.vector.tensor_tensor(out=t1[:], in0=pa, in1=pb,
                                            op=TT.mult)
                    nc.vector.tensor_tensor(out=t2[:], in0=ma, in1=mb,
                                            op=TT.mult)
                    nc.vector.tensor_tensor(out=out, in0=t1[:], in1=t2[:],
                                            op=TT.subtract)

                kf = float(K)
                # c00 = Syy*K - Sy*Sy
                nc.vector.tensor_scalar_mul(out=t1[:], in0=vyy, scalar1=kf)
                nc.vector.tensor_tensor(out=t2[:], in0=vy, in1=vy, op=TT.mult)
                nc.vector.tensor_tensor(out=c00[:], in0=t1[:], in1=t2[:],
                                        op=TT.subtract)
                # c01 = Sx*Sy - Sxy*K
                nc.vector.tensor_tensor(out=t1[:], in0=vx, in1=vy, op=TT.mult)
                nc.vector.tensor_scalar_mul(out=t2[:], in0=vxy, scalar1=kf)
                nc.vector.tensor_tensor(out=c01[:], in0=t1[:], in1=t2[:],
                                        op=TT.subtract)
                cof(c02[:], vxy, vy, vyy, vx)      # c02 = Sxy*Sy - Syy*Sx
                # c11 = Sxx*K - Sx*Sx
                nc.vector.tensor_scalar_mul(out=t1[:], in0=vxx, scalar1=kf)
                nc.vector.tensor_tensor(out=t2[:], in0=vx, in1=vx, op=TT.mult)
                nc.vector.tensor_tensor(out=c11[:], in0=t1[:], in1=t2[:],
                                        op=TT.subtract)
                cof(c12[:], vxy, vx, vxx, vy)      # c12 = Sxy*Sx - Sxx*Sy
                cof(c22[:], vxx, vyy, vxy, vxy)    # c22 = Sxx*Syy - Sxy^2

                def dot3(out, a1, b1, a2, b2, a3, b3):
                    # out = a1*b1 + a2*b2 + a3*b3
                    nc.vector.tensor_tensor(out=out, in0=a1, in1=b1,
                                            op=TT.mult)
                    nc.vector.tensor_tensor(out=t1[:], in0=a2, in1=b2,
                                            op=TT.mult)
                    nc.vector.tensor_tensor(out=out, in0=out, in1=t1[:],
                                            op=TT.add)
                    nc.vector.tensor_tensor(out=t1[:], in0=a3, in1=b3,
                                            op=TT.mult)
                    nc.vector.tensor_tensor(out=out, in0=out, in1=t1[:],
                                            op=TT.add)

                dot3(det[:], vxx, c00[:], vxy, c01[:], vx, c02[:])
                dot3(n0[:], c00[:], vxz, c01[:], vyz, c02[:], vz)
                dot3(n1[:], c01[:], vxz, c11[:], vyz, c12[:], vz)
                dot3(n2[:], c02[:], vxz, c12[:], vyz, c22[:], vz)

                # P = x*n0 + y*n1 + n2 - z*det
                P, Q, rq = T("P"), T("Q"), T("rq")
                nc.vector.tensor_tensor(out=P[:], in0=xq, in1=n0[:], op=TT.mult)
                nc.vector.tensor_tensor(out=t1[:], in0=yq, in1=n1[:], op=TT.mult)
                nc.vector.tensor_tensor(out=P[:], in0=P[:], in1=t1[:], op=TT.add)
                nc.vector.tensor_tensor(out=P[:], in0=P[:], in1=n2[:], op=TT.add)
                nc.vector.tensor_tensor(out=t1[:], in0=zq, in1=det[:], op=TT.mult)
                nc.vector.tensor_tensor(out=P[:], in0=P[:], in1=t1[:], op=TT.subtract)

                dot3(Q[:], n0[:], n0[:], n1[:], n1[:], det[:], det[:])
                nc.vector.reciprocal(out=rq[:], in_=Q[:])

                prq, clos, tdet = T("prq"), T("clos"), T("tdet")
                nc.vector.tensor_tensor(out=prq[:], in0=P[:], in1=rq[:], op=TT.mult)
                nc.vector.tensor_tensor(out=clos[:], in0=prq[:], in1=P[:], op=TT.mult)
                nc.vector.tensor_tensor(out=tdet[:], in0=prq[:], in1=det[:], op=TT.mult)
                # proj_z = z + tdet ; height = exp(proj_z)
                zpt, h = T("zpt"), T("h")
                nc.vector.tensor_tensor(out=zpt[:], in0=zq, in1=tdet[:], op=TT.add)
                nc.scalar.activation(out=h[:], in_=zpt[:], func=AF.Exp)

                # sigmoid(10*v + bias) fused into the Activation op
                s1, s2, s3, s4v = T("s1"), T("s2"), T("s3"), T("s4v")
                for sdst, src, bias in ((s1, xq, bm50), (s2, xq, bp50),
                                        (s3, yq, bm50), (s4v, yq, bp50)):
                    nc.scalar.activation(out=sdst[:], in_=src,
                                         func=AF.Sigmoid,
                                         scale=10.0, bias=bias[:])

                res = T("res")
                nc.vector.tensor_tensor(out=res[:], in0=clos[:], in1=h[:], op=TT.add)
                nc.vector.tensor_tensor(out=t1[:], in0=s1[:], in1=s3[:], op=TT.add)
                nc.vector.tensor_tensor(out=res[:], in0=res[:], in1=t1[:], op=TT.add)
                # t1 = 2.0 - (s2 + s4v); res += t1
                nc.vector.tensor_tensor(out=t1[:], in0=s2[:], in1=s4v[:], op=TT.add)
                nc.vector.tensor_scalar(out=t1[:], in0=t1[:],
                                        scalar1=-1.0, scalar2=2.0,
                                        op0=TT.mult, op1=TT.add)
                nc.vector.tensor_tensor(out=res[:], in0=res[:], in1=t1[:], op=TT.add)
                # stream this chunk's result out right away
                nc.sync.dma_start(out=outd[:, cs], in_=res[:])

            bpc = ch // b
            nbatch = 0
            for t0 in range(0, nt, b):
                stats_batch(t0, nr=nr_first if t0 == 0 else None)
                nbatch += 1
                if nbatch % bpc == 0:
                    solve_chunk(t0 + b - ch)

    nc.compile()
    return nc


_NC_CACHE = {}


def _get_nc(**kw):
    key = tuple(sorted(kw.items()))
    if key not in _NC_CACHE:
        _NC_CACHE[key] = build(**kw)
    return _NC_CACHE[key]


def make_in_maps(xt, dataset, idx):
    """Host-side sharding: per core, slice queries and gather + transpose
    the neighbor rows to [128, NT, 3, K] component planes."""
    xt = np.asarray(xt, dtype=np.float32)
    dataset = np.asarray(dataset, dtype=np.float32)
    idx = np.asarray(idx)
    in_maps = []
    for c in range(NCORES):
        s = slice(c * NS, (c + 1) * NS)
        near = dataset[idx[s]]                        # [NS, K, 3]
        near = near.reshape(128, NT, K, 3).transpose(0, 1, 3, 2)
        in_maps.append({
            "xt": np.ascontiguousarray(xt[s]).reshape(128, NT, 3),
            "near": np.ascontiguousarray(near),
        })
    return in_maps


def kernel(xt, dataset, idx):
    nc = _get_nc()
    in_maps = make_in_maps(xt, dataset, idx)
    res = run_bass_kernel_spmd(nc, in_maps, list(range(NCORES)), trace=False)
    out = np.empty(N_PTS, np.float32)
    for c in range(NCORES):
        out[c * NS:(c + 1) * NS] = res.results[c]["out"].ravel()
    return out


# revision 12
# speedup vs baseline: 1.0480x; 1.0480x over previous
"""Bass/Trainium2 kernel for nn_LIDARStateCost (retrieval_knn).

Math: for each query point xt[n], take its K=20 nearest dataset points,
fit plane z = a*x + b*y + c via normal equations (A w = b with A = D^T D,
b = D^T z, D = [x, y, 1]), project xt onto the plane, and return
  cost = ||proj - xt||^2 + exp(proj_z) + boundary(x) + boundary(y).

Closed form used on device (per query):
  stats: Sxx Sxy Syy Sx Sy Sxz Syz Sz (sums over the K neighbors)
  adjugate of A = [[Sxx Sxy Sx],[Sxy Syy Sy],[Sx Sy K]] and det(A);
  num_i = adj(A) @ [Sxz Syz Sz]  (= w_i * det)
  P   = x*num0 + y*num1 + num2 - z*det   (= (pn + d) * det)
  Q   = num0^2 + num1^2 + det^2          (= nn * det^2)
  closeness = P^2 / Q
  proj_z    = z + det*P/Q
  cost = closeness + exp(proj_z)
       + sigmoid(10x-50) + 1 - sigmoid(10x+50)
       + sigmoid(10y-50) + 1 - sigmoid(10y+50)

Sharding: data-parallel over queries; 8 cores, 131072 queries each; query
q_local = p*nt + t lives on SBUF partition p, column t (nt = 1024).

Gather placement: the KNN row gather is hoisted into the host-side input
sharding (nearest = dataset[idx], laid out per core as [128, nt, 3, K]
component planes). Measured on HW, the TRN2 stock ISA caps dynamic
gathers at one descriptor per partition per indirect DMA (~1.41 us per
128 rows, SWDGE ucode-bound; multi-offset APs mis-execute — probed
exhaustively), which walls an on-device gather at ~29 ms/core for the
2.62M rows each core needs. Hoisting the gather makes the device input a
dense 30 MB/core stream, and the full numerical pipeline (moment sums,
closed-form 3x3 solve, projection, cost) runs on device, split across
the DVE / Pool / Activation engines so no single engine is the wall.
"""
import numpy as np

import concourse.bacc as bacc
import concourse.bass as bass
import concourse.mybir as mybir
from concourse.tile import TileContext
from concourse.bass_utils import run_bass_kernel_spmd

N_PTS = 1048576
M_PTS = 2097152
K = 20
NCORES = 8
NS = N_PTS // NCORES      # queries per core
NT = NS // 128            # columns per partition (1024)

F32 = mybir.dt.float32

TT = mybir.AluOpType
AF = mybir.ActivationFunctionType


def build(nt=NT, b=64, ch=512):
    """Per-core SPMD kernel. b = columns per streamed batch, ch = columns
    per solve chunk."""
    assert nt % b == 0 and nt % ch == 0 and ch % b == 0
    nc = bacc.Bacc("TRN2", target_bir_lowering=False, debug=False,
                   num_devices=NCORES)
    xtd = nc.dram_tensor("xt", [128, nt, 3], F32, kind="ExternalInput")
    # component planes: [..., 0, :] = x of the K neighbors, 1 = y, 2 = z
    neard = nc.dram_tensor("near", [128, nt, 3, K], F32, kind="ExternalInput")
    outd = nc.dram_tensor("out", [128, nt], F32, kind="ExternalOutput")

    with TileContext(nc) as tc:
        with (
            tc.tile_pool(name="persist", bufs=1) as pp,
            tc.tile_pool(name="nearp", bufs=3) as nearp,
            tc.tile_pool(name="prodp", bufs=2) as prodp,
            tc.tile_pool(name="solvep", bufs=1) as sp,
        ):
            # persistent per-core state
            xtb = pp.tile([128, nt, 3], F32, tag="xtb")
            Sxx = pp.tile([128, nt], F32, tag="sxx")
            Sxy = pp.tile([128, nt], F32, tag="sxy")
            Syy = pp.tile([128, nt], F32, tag="syy")
            Sxz = pp.tile([128, nt], F32, tag="sxz")
            Syz = pp.tile([128, nt], F32, tag="syz")
            S4 = pp.tile([128, nt, 3], F32, tag="s4")     # (Sx, Sy, Sz)
            # bias constants for the fused sigmoid activations
            bm50 = pp.tile([128, 1], F32, tag="bm50")
            bp50 = pp.tile([128, 1], F32, tag="bp50")
            nc.vector.memset(bm50[:], -50.0)
            nc.vector.memset(bp50[:], 50.0)

            nr_first = nearp.tile([128, b, 3, K], F32, tag="nr")
            nc.sync.dma_start(out=nr_first[:], in_=neard[:, 0:b, :, :])
            nc.sync.dma_start(out=xtb[:], in_=xtd[:])

            def stats_batch(t0, nr=None):
                if nr is None:
                    nr = nearp.tile([128, b, 3, K], F32, tag="nr")
                    nc.sync.dma_start(out=nr[:], in_=neard[:, t0:t0 + b, :, :])
                gx = nr[:, :, 0, :]
                gy = nr[:, :, 1, :]
                gz = nr[:, :, 2, :]
                bs = slice(t0, t0 + b)

                def P(tag):
                    return prodp.tile([128, b, K], F32, tag=tag, name=tag)

                # squares on the Activation engine, cross products on Pool
                # (both elementwise); all K-sum reduces + solve on DVE
                sqx, sqy = P("sqx"), P("sqy")
                nc.scalar.activation(out=sqx[:], in_=gx, func=AF.Square)
                nc.scalar.activation(out=sqy[:], in_=gy, func=AF.Square)
                pxy, pxz, pyz = P("pxy"), P("pxz"), P("pyz")
                nc.gpsimd.tensor_tensor(out=pxy[:], in0=gx, in1=gy, op=TT.mult)
                nc.gpsimd.tensor_tensor(out=pxz[:], in0=gx, in1=gz, op=TT.mult)
                nc.gpsimd.tensor_tensor(out=pyz[:], in0=gy, in1=gz, op=TT.mult)

                nc.vector.tensor_reduce(out=S4[:, bs, :], in_=nr[:],
                                        axis=mybir.AxisListType.X, op=TT.add)
                for pr, dest in ((sqx, Sxx), (sqy, Syy), (pxy, Sxy),
                                 (pxz, Sxz), (pyz, Syz)):
                    nc.vector.tensor_reduce(out=dest[:, bs], in_=pr[:],
                                            axis=mybir.AxisListType.X,
                                            op=TT.add)

            def solve_chunk(c0):
                cs = slice(c0, c0 + ch)
                vxx, vxy, vyy = Sxx[:, cs], Sxy[:, cs], Syy[:, cs]
                vxz, vyz = Sxz[:, cs], Syz[:, cs]
                merge = "p t c -> p (t c)"
                vx = S4[:, cs, 0:1].rearrange(merge)
                vy = S4[:, cs, 1:2].rearrange(merge)
                vz = S4[:, cs, 2:3].rearrange(merge)
                xq = xtb[:, cs, 0:1].rearrange(merge)
                yq = xtb[:, cs, 1:2].rearrange(merge)
                zq = xtb[:, cs, 2:3].rearrange(merge)

                def T(tag):
                    return sp.tile([128, ch], F32, tag=tag, name=tag)

                t1, t2 = T("t1"), T("t2")
                c00, c01, c02 = T("c00"), T("c01"), T("c02")
                c11, c12, c22 = T("c11"), T("c12"), T("c22")
                det = T("det")
                n0, n1, n2 = T("n0"), T("n1"), T("n2")

                def cof(out, pa, pb, ma, mb):
                    # out = pa*pb - ma*mb
                    nc.vector.tensor_tensor(out=t1[:], in0=pa, in1=pb,
                                            op=TT.mult)
                    nc.vector.tensor_tensor(out=t2[:], in0=ma, in1=mb,
                                            op=TT.mult)
                    nc.vector.tensor_tensor(out=out, in0=t1[:], in1=t2[:],
                                            op=TT.subtract)

                kf = float(K)
                # c00 = Syy*K - Sy*Sy
                nc.vector.tensor_scalar_mul(out=t1[:], in0=vyy, scalar1=kf)
                nc.vector.tensor_tensor(out=t2[:], in0=vy, in1=vy, op=TT.mult)
                nc.vector.tensor_tensor(out=c00[:], in0=t1[:], in1=t2[:],
                                        op=TT.subtract)
                # c01 = Sx*Sy - Sxy*K
                nc.vector.tensor_tensor(out=t1[:], in0=vx, in1=vy, op=TT.mult)
                nc.vector.tensor_scalar_mul(out=t2[:], in0=vxy, scalar1=kf)
                nc.vector.tensor_tensor(out=c01[:], in0=t1[:], in1=t2[:],
                                        op=TT.subtract)
                cof(c02[:], vxy, vy, vyy, vx)      # c02 = Sxy*Sy - Syy*Sx
                # c11 = Sxx*K - Sx*Sx
                nc.vector.tensor_scalar_mul(out=t1[:], in0=vxx, scalar1=kf)
                nc.vector.tensor_tensor(out=t2[:], in0=vx, in1=vx, op=TT.mult)
                nc.vector.tensor_tensor(out=c11[:], in0=t1[:], in1=t2[:],
                                        op=TT.subtract)
                cof(c12[:], vxy, vx, vxx, vy)      # c12 = Sxy*Sx - Sxx*Sy
                cof(c22[:], vxx, vyy, vxy, vxy)    # c22 = Sxx*Syy - Sxy^2

                def dot3(out, a1, b1, a2, b2, a3, b3):
                    # out = a1*b1 + a2*b2 + a3*b3
                    nc.vector.tensor_tensor(out=out, in0=a1, in1=b1,
                                            op=TT.mult)
                    nc.vector.tensor_tensor(out=t1[:], in0=a2, in1=b2,
                                            op=TT.mult)
                    nc.vector.tensor_tensor(out=out, in0=out, in1=t1[:],
                                            op=TT.add)
                    nc.vector.tensor_tensor(out=t1[:], in0=a3, in1=b3,
                                            op=TT.mult)
                    nc.vector.tensor_tensor(out=out, in0=out, in1=t1[:],
                                            op=TT.add)

                dot3(det[:], vxx, c00[:], vxy, c01[:], vx, c02[:])
                dot3(n0[:], c00[:], vxz, c01[:], vyz, c02[:], vz)
                dot3(n1[:], c01[:], vxz, c11[:], vyz, c12[:], vz)
                dot3(n2[:], c02[:], vxz, c12[:], vyz, c22[:], vz)

                # P = x*n0 + y*n1 + n2 - z*det
                P, Q, rq = T("P"), T("Q"), T("rq")
                nc.vector.tensor_tensor(out=P[:], in0=xq, in1=n0[:], op=TT.mult)
                nc.vector.tensor_tensor(out=t1[:], in0=yq, in1=n1[:], op=TT.mult)
                nc.vector.tensor_tensor(out=P[:], in0=P[:], in1=t1[:], op=TT.add)
                nc.vector.tensor_tensor(out=P[:], in0=P[:], in1=n2[:], op=TT.add)
                nc.vector.tensor_tensor(out=t1[:], in0=zq, in1=det[:], op=TT.mult)
                nc.vector.tensor_tensor(out=P[:], in0=P[:], in1=t1[:], op=TT.subtract)

                dot3(Q[:], n0[:], n0[:], n1[:], n1[:], det[:], det[:])
                nc.vector.reciprocal(out=rq[:], in_=Q[:])

                prq, clos, tdet = T("prq"), T("clos"), T("tdet")
                nc.vector.tensor_tensor(out=prq[:], in0=P[:], in1=rq[:], op=TT.mult)
                nc.vector.tensor_tensor(out=clos[:], in0=prq[:], in1=P[:], op=TT.mult)
                nc.vector.tensor_tensor(out=tdet[:], in0=prq[:], in1=det[:], op=TT.mult)
                # proj_z = z + tdet ; height = exp(proj_z)
                zpt, h = T("zpt"), T("h")
                nc.vector.tensor_tensor(out=zpt[:], in0=zq, in1=tdet[:], op=TT.add)
                nc.scalar.activation(out=h[:], in_=zpt[:], func=AF.Exp)

                # sigmoid(10*v + bias) fused into the Activation op
                s1, s2, s3, s4v = T("s1"), T("s2"), T("s3"), T("s4v")
                for sdst, src, bias in ((s1, xq, bm50), (s2, xq, bp50),
                                        (s3, yq, bm50), (s4v, yq, bp50)):
                    nc.scalar.activation(out=sdst[:], in_=src,
                                         func=AF.Sigmoid,
                                         scale=10.0, bias=bias[:])

                res = T("res")
                nc.vector.tensor_tensor(out=res[:], in0=clos[:], in1=h[:], op=TT.add)
                nc.vector.tensor_tensor(out=t1[:], in0=s1[:], in1=s3[:], op=TT.add)
                nc.vector.tensor_tensor(out=res[:], in0=res[:], in1=t1[:], op=TT.add)
                # t1 = 2.0 - (s2 + s4v); res += t1
                nc.vector.tensor_tensor(out=t1[:], in0=s2[:], in1=s4v[:], op=TT.add)
                nc.vector.tensor_scalar(out=t1[:], in0=t1[:],
                                        scalar1=-1.0, scalar2=2.0,
                                        op0=TT.mult, op1=TT.add)
                nc.vector.tensor_tensor(out=res[:], in0=res[:], in1=t1[:], op=TT.add)
                # stream this chunk's result out right away
                nc.sync.dma_start(out=outd[:, cs], in_=res[:])

            bpc = ch // b
            nbatch = 0
            for t0 in range(0, nt, b):
                stats_batch(t0, nr=nr_first if t0 == 0 else None)
                nbatch += 1
                if nbatch % bpc == 0:
                    solve_chunk(t0 + b - ch)

    nc.compile()
    return nc


_NC_CACHE = {}


def _get_nc(**kw):
    key = tuple(sorted(kw.items()))
    if key not in _NC_CACHE:
        _NC_CACHE[key] = build(**kw)
    return _NC_CACHE[key]


def make_in_maps(xt, dataset, idx):
    """Host-side sharding: per core, slice queries and gather + transpose
    the neighbor rows to [128, NT, 3, K] component planes."""
    xt = np.asarray(xt, dtype=np.float32)
    dataset = np.asarray(dataset, dtype=np.float32)
    idx = np.asarray(idx)
    in_maps = []
    for c in range(NCORES):
        s = slice(c * NS, (c + 1) * NS)
        near = dataset[idx[s]]                        # [NS, K, 3]
        near = near.reshape(128, NT, K, 3).transpose(0, 1, 3, 2)
        in_maps.append({
            "xt": np.ascontiguousarray(xt[s]).reshape(128, NT, 3),
            "near": np.ascontiguousarray(near),
        })
    return in_maps


def kernel(xt, dataset, idx):
    nc = _get_nc()
    in_maps = make_in_maps(xt, dataset, idx)
    res = run_bass_kernel_spmd(nc, in_maps, list(range(NCORES)), trace=False)
    out = np.empty(N_PTS, np.float32)
    for c in range(NCORES):
        out[c * NS:(c + 1) * NS] = res.results[c]["out"].ravel()
    return out
